# revision 23
# baseline (speedup 1.0000x reference)
"""Trainium2 Bass kernel for BasicSelfAttention2D (spatial-reduction attention).

Reference computation (per image):
    q   = (wq @ x_flat)              [d=32, N=4096]
    xkv = avgpool2x2(x)              [C, Nk=1024]
    k   = wk @ xkv                   [d, Nk]
    v   = wv @ xkv                   [C, Nk]
    attn= softmax(q^T k / sqrt(d))   [N, Nk]
    out = v @ attn^T                 [C, N]
    y   = x + gamma * (wo @ out)

Sharding: data-parallel over batch, one image per NeuronCore (8 cores).

Kernel strategy (per core):
  - scores are built TRANSPOSED  s_T[m, n]  (m = kv position on partitions)
    so that exp(s_T) can directly feed the attention-aggregation matmul
    (which contracts over m on the partition dim).  The softmax denominator
    rsum[n] = sum_m exp(s_T[m,n]) is computed with a DVE add-tree +
    GPSIMD partition_all_reduce, and its reciprocal is applied as a
    per-column scale where it commutes with the out-projection
    (folded into the PSUM->SBUF copy of out_u, before out-proj).
  - q/k projections are emitted 2x column-packed (tile_position) so q,k are
    REPLICATED across two 32-partition groups; score matmuls (K=32) are then
    2-way row-packed, and the next super's score packs are interleaved with
    the current super's aggregation matmuls to keep the in-order PE queue fed.
  - all matmuls are bf16 (1 cyc/row; fp32 is 4x slower and fp32r's fused
    weight-load path only tolerates a single sync-wait in walrus codegen).
    Independent bf16 rounding errors do not amplify through reductions.
    The residual add x + (.) happens in fp32 against fp32 PSUM.
  - host-side prep folds: 1/4 (avgpool mean) into wkT/wvT, gamma into woT,
    and provides bf16 copies of x / weights (layout+dtype prep only).
"""

import ml_dtypes
import numpy as np

import concourse.bacc as bacc
import concourse.mybir as mybir
from concourse.tile import TileContext
from concourse.bass_utils import run_bass_kernel_spmd

B, C, H, W = 8, 256, 64, 64
N = H * W          # 4096
D = 32             # q/k dim
NK = (H // 2) * (W // 2)   # 1024
NCORES = 8

F32 = mybir.dt.float32
F16 = mybir.dt.float16
BF16 = mybir.dt.bfloat16

SCALE = 1.0 / np.sqrt(np.float32(D))   # softmax scale

SUP = 1024          # n-super width (exp granularity)
NSUP = N // SUP     # 4
NCHUNK = 512        # matmul free-dim chunk
MT = NK // 128      # 8 m-tiles


def build_nc():
    nc = bacc.Bacc(None, target_bir_lowering=False, debug=False)

    x_in = nc.dram_tensor("x", [C, N], F32, kind="ExternalInput")
    xb_in = nc.dram_tensor("xb", [C, N], BF16, kind="ExternalInput")
    WPACK = D + D + C + C   # wqT | wkT | wvT | woT  along the free dim
    wall_in = nc.dram_tensor("wall", [C, WPACK], BF16, kind="ExternalInput")
    y_out = nc.dram_tensor("y", [C, N], F32, kind="ExternalOutput")

    with TileContext(nc) as tc:
        with (
            tc.tile_pool(name="big", bufs=1) as big,
            tc.tile_pool(name="work", bufs=2) as work,
            tc.tile_pool(name="etp", bufs=3) as etp,
            tc.tile_pool(name="ystage", bufs=4) as ypool,
            tc.tile_pool(name="xres", bufs=4) as xpool,
            tc.tile_pool(name="rows", bufs=4) as rowp,
            tc.tile_pool(name="ps_sc", bufs=2, space="PSUM") as ps_sc,
            tc.tile_pool(name="ps_ac", bufs=4, space="PSUM") as ps_ac,
        ):
            # ---------------- persistent SBUF ----------------
            xb_sb = big.tile([128, 2, N], BF16, tag="xb")     # c-half major
            xkv_sb = big.tile([128, 2, NK], BF16, tag="xkv")
            qrep_sb = big.tile([128, N], BF16, tag="qrep")    # q replicated 4x
            krep_sb = big.tile([128, NK], BF16, tag="krep")   # k replicated 4x
            vT_sb = big.tile([128, MT, C], BF16, tag="vT")    # v transposed
            # DMA staging for weights; the ACT copy into the real tile makes
            # every matmul weight-dependency an ACT-sem wait (merges with the
            # other ACT waits -- walrus caps matmuls at 2 sync waits).
            w_st = big.tile([128, 2, WPACK], BF16, tag="w_st")
            w_sb = big.tile([128, 2, WPACK], BF16, tag="w_sb")
            wq_sb = w_sb[:, :, 0:D]
            wk_sb = w_sb[:, :, D : 2 * D]
            wv_sb = w_sb[:, :, 2 * D : 2 * D + C]
            wo_sb = w_sb[:, :, 2 * D + C :].rearrange("p t (o k) -> p t o k", o=2)

            # ---------------- input DMAs ----------------
            nc.sync.dma_start(
                out=w_st, in_=wall_in.rearrange("(t p) w -> p t w", p=128)
            )
            for s in range(NSUP):
                nsl = slice(s * SUP, (s + 1) * SUP)
                for ch in range(2):
                    rows = slice(ch * 128, (ch + 1) * 128)
                    nc.sync.dma_start(out=xb_sb[:, ch, nsl], in_=xb_in[rows, nsl])
            nc.scalar.activation(
                out=w_sb, in_=w_st, func=mybir.ActivationFunctionType.Copy
            )
            # ones vectors for the softmax-denominator matmuls
            ones_col = big.tile([128, 1], BF16, tag="ones_col")
            nc.vector.memset(ones_col, 1.0)
            ones_row = big.tile([1, 128], F16, tag="ones_row")
            nc.vector.memset(ones_row, 1.0)
            # dummy exp: pulls the ACT exp table load into the setup phase
            warm = big.tile([128, 1], F32, tag="warm")
            nc.vector.memset(warm, 0.0)
            nc.scalar.activation(
                out=warm, in_=warm, func=mybir.ActivationFunctionType.Exp
            )

            # ---------------- q projection (replicated 2x col-packed) -----
            for cn in range(N // NCHUNK):
                nsl = slice(cn * NCHUNK, (cn + 1) * NCHUNK)
                qp = ps_sc.tile([128, NCHUNK], F32, tag="sc")
                for j in range(2):
                    for ch in range(2):
                        nc.tensor.matmul(
                            qp[32 * j : 32 * (j + 1), :],
                            lhsT=wq_sb[:, ch, :],
                            rhs=xb_sb[:, ch, nsl],
                            start=(ch == 0),
                            stop=(ch == 1),
                            tile_position=(0, 32 * j),
                        )
                nc.scalar.activation(
                    out=qrep_sb[0:64, nsl], in_=qp[0:64, :],
                    func=mybir.ActivationFunctionType.Copy,
                )

            # ---------------- avgpool (sum; /4 folded into wkT/wvT) -------
            for ch in range(2):
                xw = work.tile([128, 64, 32], BF16, tag="xw")  # w-paired sums
                x4 = xb_sb[:, ch, :].rearrange("p (h w t) -> p h w t", h=64, w=32)
                for s in range(NSUP):
                    hs = slice(s * 16, (s + 1) * 16)
                    nc.vector.tensor_add(
                        out=xw[:, hs, :], in0=x4[:, hs, :, 0], in1=x4[:, hs, :, 1]
                    )
                xh = xw.rearrange("p (h2 t) w -> p h2 t w", t=2)
                xkv_v = xkv_sb[:, ch, :].rearrange("p (a b) -> p a b", a=32)
                for s in range(NSUP):
                    h2s = slice(s * 8, (s + 1) * 8)
                    nc.vector.tensor_add(
                        out=xkv_v[:, h2s, :],
                        in0=xh[:, h2s, 0, :],
                        in1=xh[:, h2s, 1, :],
                    )

            # ---------------- k projection (replicated 2x col-packed) -----
            for cn in range(NK // NCHUNK):
                nsl = slice(cn * NCHUNK, (cn + 1) * NCHUNK)
                kp = ps_sc.tile([128, NCHUNK], F32, tag="sc")
                for j in range(2):
                    for ch in range(2):
                        nc.tensor.matmul(
                            kp[32 * j : 32 * (j + 1), :],
                            lhsT=wk_sb[:, ch, :],
                            rhs=xkv_sb[:, ch, nsl],
                            start=(ch == 0),
                            stop=(ch == 1),
                            tile_position=(0, 32 * j),
                        )
                nc.scalar.activation(
                    out=krep_sb[0:64, nsl], in_=kp[0:64, :],
                    func=mybir.ActivationFunctionType.Copy,
                )

            # ---------------- v projection (transposed) ----------------
            for mt in range(MT):
                msl = slice(mt * 128, (mt + 1) * 128)
                vp = ps_ac.tile([128, C], F32, tag="ac")
                for ch in range(2):
                    nc.tensor.matmul(
                        vp,
                        lhsT=xkv_sb[:, ch, msl],
                        rhs=wv_sb[:, ch, :],
                        start=(ch == 0),
                        stop=(ch == 1),
                    )
                nc.scalar.activation(
                    out=vT_sb[:, mt, :], in_=vp,
                    func=mybir.ActivationFunctionType.Copy,
                )

            # ---------------- main loop over n-supers ----------------
            def make_pack_ops(s):
                """Return 4 closures; each emits one 2-way-packed score pack
                (2 matmuls + 2 exps + 1 stage-1 add) for super s."""
                et = etp.tile([128, MT, SUP], BF16, tag="et", name="et")
                p4 = work.tile([128, 4, SUP], BF16, tag="p4", name="p4")

                def pack(mp):
                    sc_ps = [
                        ps_sc.tile([128, SUP], F32, tag="sc", name=f"sc{i}")
                        for i in range(2)
                    ]
                    for half in range(2):
                        hsl = slice(s * SUP + half * NCHUNK,
                                    s * SUP + (half + 1) * NCHUNK)
                        osl = slice(half * NCHUNK, (half + 1) * NCHUNK)
                        for i in range(2):
                            mt = 2 * mp + i
                            base = slice(32 * i, 32 * (i + 1))
                            nc.tensor.matmul(
                                sc_ps[i][:, osl],
                                lhsT=krep_sb[base, mt * 128 : (mt + 1) * 128],
                                rhs=qrep_sb[base, hsl],
                                tile_position=(32 * i, 0),
                            )
                    for i in range(2):
                        nc.scalar.activation(
                            out=et[:, 2 * mp + i, :], in_=sc_ps[i],
                            func=mybir.ActivationFunctionType.Exp,
                            scale=float(SCALE),
                        )
                    nc.vector.tensor_add(
                        out=p4[:, mp, :], in0=et[:, 2 * mp, :],
                        in1=et[:, 2 * mp + 1, :],
                    )

                return et, p4, [lambda mp=mp: pack(mp) for mp in range(4)]

            cur = make_pack_ops(0)
            for op in cur[2]:
                op()
            for s in range(NSUP):
                et, p4, _ = cur
                nxt_packs = []
                if s + 1 < NSUP:
                    cur = make_pack_ops(s + 1)
                    nxt_packs = list(cur[2])

                def next_pack():
                    if nxt_packs:
                        nxt_packs.pop(0)()

                # denominator tree stages 2+3 -> single partial p1
                p2 = work.tile([128, 2, SUP], BF16, tag="p2")
                nc.vector.tensor_add(out=p2[:, 0, :], in0=p4[:, 0, :], in1=p4[:, 1, :])
                nc.vector.tensor_add(out=p2[:, 1, :], in0=p4[:, 2, :], in1=p4[:, 3, :])
                p1 = work.tile([128, SUP], BF16, tag="p1")
                nc.vector.tensor_add(out=p1, in0=p2[:, 0, :], in1=p2[:, 1, :])

                # denominator finish: ones-matmul row-sum, fp16 row copy,
                # K=1 broadcast matmul, fast reciprocal into SBUF
                outu = work.tile([128, 2, SUP], BF16, tag="outu")
                av_ps = {}
                for c in range(2):
                    for half in range(2):
                        av_ps[c, half] = ps_ac.tile(
                            [128, NCHUNK], F32, tag="ac", name=f"av{c}{half}"
                        )
                scale_sb = {}
                for half in range(2):
                    osl = slice(half * NCHUNK, (half + 1) * NCHUNK)
                    rs_ps = ps_ac.tile([1, NCHUNK], F32, tag="ac", name="rs")
                    nc.tensor.matmul(rs_ps, lhsT=ones_col, rhs=p1[:, osl])
                    rs_row = rowp.tile([1, NCHUNK], F16, tag="rs_row")
                    nc.scalar.activation(
                        out=rs_row, in_=rs_ps,
                        func=mybir.ActivationFunctionType.Copy,
                    )
                    bc_ps = ps_ac.tile([128, NCHUNK], F32, tag="ac", name="bc")
                    nc.tensor.matmul(bc_ps, lhsT=ones_row, rhs=rs_row)
                    sc_t = rowp.tile([128, NCHUNK], F32, tag="scale")
                    nc.vector.reciprocal_approx_fast(out=sc_t, in_=bc_ps)
                    scale_sb[half] = sc_t

                # attention aggregation (contract m), interleaved with the
                # NEXT super's score packs so the PE queue always has ready
                # work while ACT pipelines the exps
                for c in range(2):
                    for mt in range(MT):
                        if mt % 2 == 0:
                            next_pack()
                        for half in range(2):
                            osl = slice(half * NCHUNK, (half + 1) * NCHUNK)
                            nc.tensor.matmul(
                                av_ps[c, half],
                                lhsT=vT_sb[:, mt, c * 128 : (c + 1) * 128],
                                rhs=et[:, mt, osl],
                                start=(mt == 0),
                                stop=(mt == MT - 1),
                            )
                    for half in range(2):
                        osl = slice(half * NCHUNK, (half + 1) * NCHUNK)
                        nc.vector.scalar_tensor_tensor(
                            out=outu[:, c, osl],
                            in0=av_ps[c, half],
                            scalar=1.0,
                            in1=scale_sb[half],
                            op0=mybir.AluOpType.mult,
                            op1=mybir.AluOpType.mult,
                        )
                while nxt_packs:
                    next_pack()

                # out-projection + residual add (fp32) + store
                for half in range(2):
                    osl = slice(half * NCHUNK, (half + 1) * NCHUNK)
                    fsl = slice(s * SUP + half * NCHUNK,
                                s * SUP + (half + 1) * NCHUNK)
                    for ot in range(2):
                        xres = xpool.tile([128, NCHUNK], F32, tag="xr")
                        nc.gpsimd.dma_start(
                            out=xres, in_=x_in[ot * 128 : (ot + 1) * 128, fsl]
                        )
                        op_ps = ps_ac.tile([128, NCHUNK], F32, tag="ac", name="op")
                        for ch in range(2):
                            nc.tensor.matmul(
                                op_ps,
                                lhsT=wo_sb[:, ch, ot, :],
                                rhs=outu[:, ch, osl],
                                start=(ch == 0),
                                stop=(ch == 1),
                            )
                        y_st = ypool.tile([128, NCHUNK], F32, tag="y")
                        nc.vector.tensor_add(out=y_st, in0=xres, in1=op_ps)
                        nc.sync.dma_start(
                            out=y_out[ot * 128 : (ot + 1) * 128, fsl], in_=y_st
                        )
    nc.compile()
    return nc


_NC_CACHE = {}


def _get_nc():
    if "nc" not in _NC_CACHE:
        _NC_CACHE["nc"] = build_nc()
    return _NC_CACHE["nc"]


def _prep_inputs(x, wq, wk, wv, wo, gamma):
    bf = ml_dtypes.bfloat16
    x = np.ascontiguousarray(np.asarray(x, dtype=np.float32))
    xb = x.astype(bf)
    wqT = np.asarray(wq, np.float32).T
    wkT = np.asarray(wk, np.float32).T * 0.25
    wvT = np.asarray(wv, np.float32).T * 0.25
    woT = np.float32(np.asarray(gamma, np.float32)[0]) * np.asarray(wo, np.float32).T
    wall = np.ascontiguousarray(
        np.concatenate([wqT, wkT, wvT, woT], axis=1)
    ).astype(bf)
    in_maps = []
    for i in range(NCORES):
        in_maps.append({
            "x": np.ascontiguousarray(x[i].reshape(C, N)),
            "xb": np.ascontiguousarray(xb[i].reshape(C, N)),
            "wall": wall,
        })
    return in_maps


def run(x, wq, wk, wv, wo, gamma, trace=False, **trace_kwargs):
    nc = _get_nc()
    in_maps = _prep_inputs(x, wq, wk, wv, wo, gamma)
    res = run_bass_kernel_spmd(
        nc, in_maps, list(range(NCORES)), trace=trace, **trace_kwargs
    )
    y = np.stack([res.results[i]["y"].reshape(C, H, W) for i in range(NCORES)])
    return y, res


def kernel(x, wq, wk, wv, wo, gamma):
    y, _ = run(x, wq, wk, wv, wo, gamma, trace=False)
    return y
